# revision 1
# baseline (speedup 1.0000x reference)
"""Trainium2 Bass kernel v2 for NeatModule forward (gnn_message_passing).

All-dense bf16 design, 8 cores SPMD, 4 AllGathers per rep, no indirect DMA:
  - l0 REPLICATED dense, orientation A (dst rows on PSUM partitions) from
    SBUF x tiles; activations+masks write the SBUF source region reg0.
  - l1..l4 SHARDED dense, orientation B: stationary = source chunk
    [128 src, 128 batch], moving = W block [128 src, cols]; PSUM is
    [128 batch, cols]. Activations with per-COLUMN masks, then PE
    transposes back to [dst, batch], bf16 copies, contrib DMA, AllGather
    into a Shared DRAM tensor, then one DMA into the SBUF region for
    later layers' stationaries.
  - l4's AllGather feeds only l5. l5 is REPLICATED dense (256 output
    nodes), fp32 activations, PE-transposed straight into `out`.
  - W0,W1,W2,W4 are SBUF-resident; W3 and W5 stream from HBM per rep
    through small rotating buffers (no deps -> fully overlapped).
  - Matmuls for chunks whose sources are already available are emitted
    BEFORE the collective they do not depend on, so the PE works through
    each AllGather; reps pipeline through the collective chain.
"""
import numpy as np
import ml_dtypes

import concourse.bass as bass
import concourse.mybir as mybir
import concourse.tile as tile
from concourse.vector_clock import ScopedClock, VectorClock
from concourse.tile_rust import add_dep_helper
from concourse.bass_utils import run_bass_kernel_spmd

NUM_INPUTS = 512
NUM_OUTPUTS = 256
NUM_NODES = 20000
NUM_LAYERS = 6
CHUNK = (NUM_NODES - NUM_INPUTS) // NUM_LAYERS  # 3248
NCORES = 8
P = 128
REG = 3328                   # padded per-layer slot count (l0..l3), 26 chunks
SL = 416                     # per-core dst slots l1..l3
L4_N = 1280                  # l4 compact slots (10 chunks)
SL4 = 160                    # per-core dst slots l4
L4LO = NUM_INPUTS + 4 * CHUNK    # 13504
L5LO = NUM_INPUTS + 5 * CHUNK    # 16752
OUT0 = NUM_NODES - NUM_OUTPUTS   # 19744
BANK_W = [512] * 6 + [256]       # l0 PSUM bank tiling over 3328 slots
# source chunk counts per dense layer (l1..l5): x,l0 | +l1 | +l2 | +l3 | +l4
NCH = {1: 30, 2: 56, 3: 82, 4: 108, 5: 118}
COLS = {1: SL, 2: SL, 3: SL, 4: SL4, 5: NUM_OUTPUTS}
W3_BLK = 8                   # chunks per W3 stream block
W5_BLK = 6

bf16 = ml_dtypes.bfloat16
ABLATE_COLL = False
ABLATE_STREAM = False

# ---------------------------------------------------------------- tile fixes


def _drain_and_barrier(self, tick_clock, wait_clock):
    gc = tick_clock.global_clock
    n = len(gc)
    for p in range(n):
        t = gc[p]
        if t > 0:
            nop = self.nc.sync.nop(nofuse=True)
            vec = [0] * n
            vec[p] = t
            wait_clock.add_sem_waits(nop.ins, ScopedClock({None: VectorClock(vec)}))
    self.nc.sync.drain()
    self.nc.all_engine_barrier()
    assert self.sems is not None
    popped = self.nc._tile_sem_poison_stack.pop()
    assert popped is self._sem_poison
    self.nc.clear_and_free_semaphores(list(self.sems.allocated().values()))
    self.nc.all_engine_barrier()


tile.TileContext._drain_and_barrier = _drain_and_barrier


def split_waits(nc, K=1):
    n_split = 0
    for f in nc.m.functions:
        for bb in f.blocks:
            insts = list(bb.instructions)
            out = []
            changed = False
            for inst in insts:
                si = inst.sync_info
                if si is not None and si.on_wait is not None and len(si.on_wait) > K:
                    waits = list(si.on_wait)
                    over, keep = waits[:-K], waits[-K:]
                    for j in range(0, len(over), K):
                        out.append(mybir.InstNoOp(
                            name=f"I-waitsplit-{nc.next_id()}",
                            engine=inst.engine,
                            sync_info=mybir.SyncInfo(on_wait=over[j:j + K], on_update=[]),
                        ))
                    si.on_wait = keep
                    inst.sync_info = si
                    changed = True
                    n_split += 1
                out.append(inst)
            if changed:
                bb.instructions = out
    return n_split


# ------------------------------------------------------------------ host prep


def _prune_edges(edge_src, edge_dst):
    needed = np.zeros(NUM_NODES, dtype=bool)
    needed[NUM_NODES - NUM_OUTPUTS:] = True
    layer_of = np.full(NUM_NODES, -1)
    for l in range(NUM_LAYERS):
        layer_of[NUM_INPUTS + l * CHUNK: NUM_INPUTS + (l + 1) * CHUNK] = l
    dst_l = layer_of[edge_dst]
    keep = np.zeros(len(edge_src), dtype=bool)
    for l in range(NUM_LAYERS - 1, -1, -1):
        m = (dst_l == l) & needed[edge_dst]
        keep |= m
        needed[edge_src[m]] = True
    return keep, needed


def prep(weights, edge_src, edge_dst, act_id):
    es = np.asarray(edge_src).astype(np.int64)
    ed = np.asarray(edge_dst).astype(np.int64)
    ew = np.asarray(weights).astype(np.float32)
    act_id = np.asarray(act_id).astype(np.int64)

    keep, needed = _prune_edges(es, ed)
    es, ed, ew = es[keep], ed[keep], ew[keep]
    e_l = (ed - NUM_INPUTS) // CHUNK

    needed4 = np.where(needed[L4LO:L5LO])[0]      # rel ids of kept l4 nodes
    n4 = len(needed4)
    assert n4 <= L4_N
    c4map = np.full(CHUNK, -1, dtype=np.int64)
    c4map[needed4] = np.arange(n4)

    # l0 slot order sorted by activation class (sig | tanh | relu+pads) so the
    # replicated l0 needs only range-activations, no predicated copies.
    act0 = act_id[NUM_INPUTS:NUM_INPUTS + CHUNK]
    cls0 = np.full(REG, 2, dtype=np.int64)
    cls0[:CHUNK] = np.where(act0 == 0, 0, np.where(act0 == 1, 1, 2))
    perm0 = np.lexsort((np.arange(REG), cls0))    # slot s holds rel perm0[s]
    inv0 = np.empty(REG, dtype=np.int64)
    inv0[perm0] = np.arange(REG)
    B1 = int((cls0 == 0).sum())
    B2 = B1 + int((cls0 == 1).sum())

    # global source-chunk index of a source node (chunk-structured regions)
    def src_chunk_pos(s):
        """-> (chunk index 0..117, row within chunk)"""
        gc = np.where(s < NUM_INPUTS, s // P, 0)
        pp = np.where(s < NUM_INPUTS, s % P, 0)
        for l in range(4):
            lo = NUM_INPUTS + l * CHUNK
            m = (s >= lo) & (s < lo + CHUNK)
            rel = np.clip(s - lo, 0, CHUNK - 1)
            pos = inv0[rel] if l == 0 else rel
            gc = np.where(m, 4 + 26 * l + pos // P, gc)
            pp = np.where(m, pos % P, pp)
        m4 = (s >= L4LO) & (s < L5LO)
        if m4.any():
            cc = c4map[np.clip(s - L4LO, 0, CHUNK - 1)]
            assert (cc[m4] >= 0).all()
            gc = np.where(m4, 108 + cc // P, gc)
            pp = np.where(m4, cc % P, pp)
        return gc.astype(np.int64), pp.astype(np.int64)

    # ---------------- W0 (replicated, orientation A, act-sorted dst slots)
    W0 = np.zeros((P, 26 * 4 * P), dtype=np.float32)
    m = e_l == 0
    s, w = es[m], ew[m]
    d = inv0[ed[m] - NUM_INPUTS]
    c, sp = s // P, s % P
    t, q = d // P, d % P
    np.add.at(W0, (sp, (t * 4 + c) * P + q), w)

    # ---------------- W1..W4 (sharded, orientation B: [src_p, chunk*cols + dstcol])
    Wd = {}
    for l in (1, 2, 3, 4):
        cols = COLS[l]
        Wl = np.zeros((NCORES, P, NCH[l] * cols), dtype=np.float32)
        m = e_l == l
        s, w = es[m], ew[m]
        if l < 4:
            dd = ed[m] - (NUM_INPUTS + l * CHUNK)
        else:
            dd = c4map[ed[m] - L4LO]
            assert (dd >= 0).all()
        cor = dd // cols
        j = dd % cols
        gc, sp = src_chunk_pos(s)
        assert gc.max() < NCH[l]
        np.add.at(Wl, (cor, sp, gc * cols + j), w)
        Wd[l] = Wl

    # ---------------- W5 (replicated)
    W5 = np.zeros((P, NCH[5] * NUM_OUTPUTS), dtype=np.float32)
    m = e_l == 5
    s, w = es[m], ew[m]
    oc = ed[m] - OUT0
    assert (oc >= 0).all()
    gc, sp = src_chunk_pos(s)
    np.add.at(W5, (sp, gc * NUM_OUTPUTS + oc), w)

    # ---------------- masks
    rows = np.arange(P)
    # row masks (l0: 26 cols, l5: 2 cols), replicated
    MSr = np.zeros((P, 28), dtype=np.int8)
    MTr = np.zeros((P, 28), dtype=np.int8)
    for t in range(26):
        rel_s = perm0[t * P + rows]          # act-sorted slot order
        v = rel_s < CHUNK
        nd = NUM_INPUTS + rel_s[v]
        MSr[rows[v], t] = act_id[nd] == 0
        MTr[rows[v], t] = act_id[nd] == 1
    for t in range(2):
        nd = OUT0 + t * P + rows
        MSr[rows, 26 + t] = act_id[nd] == 0
        MTr[rows, 26 + t] = act_id[nd] == 1
    # column masks per core (l1..l4 and l5(replicated)): value per dst col,
    # broadcast down rows; layout [416 | 416 | 416 | 160 | 256] = 1664 cols
    MSc = np.zeros((NCORES, P, 1664), dtype=np.int8)
    MTc = np.zeros((NCORES, P, 1664), dtype=np.int8)
    off = {1: 0, 2: 416, 3: 832, 4: 1248, 5: 1408}
    for l in (1, 2, 3):
        base = NUM_INPUTS + l * CHUNK
        for i in range(NCORES):
            jj = np.arange(SL)
            rel = SL * i + jj
            v = rel < CHUNK
            nd = base + rel[v]
            MSc[i, :, off[l] + jj[v]] = (act_id[nd] == 0).astype(np.int8)[:, None]
            MTc[i, :, off[l] + jj[v]] = (act_id[nd] == 1).astype(np.int8)[:, None]
    for i in range(NCORES):
        jj = np.arange(SL4)
        cc = SL4 * i + jj
        v = cc < n4
        nd = L4LO + needed4[cc[v]]
        MSc[i, :, off[4] + jj[v]] = (act_id[nd] == 0).astype(np.int8)[:, None]
        MTc[i, :, off[4] + jj[v]] = (act_id[nd] == 1).astype(np.int8)[:, None]
    jj = np.arange(NUM_OUTPUTS)
    nd = OUT0 + jj
    for i in range(NCORES):
        MSc[i, :, off[5] + jj] = (act_id[nd] == 0).astype(np.int8)[:, None]
        MTc[i, :, off[5] + jj] = (act_id[nd] == 1).astype(np.int8)[:, None]

    ident = np.eye(P, dtype=np.float32)

    data = dict(
        W0=W0.astype(bf16),
        W1=Wd[1].astype(bf16), W2=Wd[2].astype(bf16),
        W3=Wd[3].astype(bf16), W4=Wd[4].astype(bf16),
        W5=W5.astype(bf16),
        MSr=MSr, MTr=MTr, MSc=MSc, MTc=MTc,
        IDb=ident.astype(bf16), IDf=ident,
    )
    return dict(B1=B1, B2=B2), data


# -------------------------------------------------------------- kernel build


def build_nc(meta, reps=1):
    f32 = mybir.dt.float32
    bf = mybir.dt.bfloat16
    i8 = mybir.dt.int8
    AF = mybir.ActivationFunctionType

    nc = bass.Bass()
    xn = nc.declare_dram_parameter("xn", [NUM_INPUTS, P], bf, isOutput=False)
    W0_in = nc.declare_dram_parameter("W0", [P, 26 * 4 * P], bf, isOutput=False)
    W1_in = nc.declare_dram_parameter("W1", [P, NCH[1] * SL], bf, isOutput=False)
    W2_in = nc.declare_dram_parameter("W2", [P, NCH[2] * SL], bf, isOutput=False)
    W3_in = nc.declare_dram_parameter("W3", [P, NCH[3] * SL], bf, isOutput=False)
    W4_in = nc.declare_dram_parameter("W4", [P, NCH[4] * SL4], bf, isOutput=False)
    W5_in = nc.declare_dram_parameter("W5", [P, NCH[5] * NUM_OUTPUTS], bf, isOutput=False)
    MSr_in = nc.declare_dram_parameter("MSr", [P, 28], i8, isOutput=False)
    MTr_in = nc.declare_dram_parameter("MTr", [P, 28], i8, isOutput=False)
    MSc_in = nc.declare_dram_parameter("MSc", [P, 1664], i8, isOutput=False)
    MTc_in = nc.declare_dram_parameter("MTc", [P, 1664], i8, isOutput=False)
    IDb_in = nc.declare_dram_parameter("IDb", [P, P], bf, isOutput=False)
    IDf_in = nc.declare_dram_parameter("IDf", [P, P], f32, isOutput=False)
    out = nc.declare_dram_parameter("out", [NUM_OUTPUTS, P], f32, isOutput=True)

    g = {l: nc.dram_tensor(f"g{l}", [REG if l < 4 else L4_N, P], bf,
                           addr_space="Shared") for l in (1, 2, 3, 4)}
    contrib = {l: nc.dram_tensor(f"c{l}", [SL if l < 4 else SL4, P], bf)
               for l in (1, 2, 3, 4)}

    COff = {1: 0, 2: 416, 3: 832, 4: 1248, 5: 1408}

    with tile.TileContext(nc) as tc:
        with (
            tc.tile_pool(name="big", bufs=1) as big,
            tc.tile_pool(name="w3b", bufs=2) as w3p,
            tc.tile_pool(name="w5b", bufs=2) as w5p,
            tc.tile_pool(name="stage", bufs=2) as stage,
            tc.tile_pool(name="ct", bufs=6) as ctp,
            tc.tile_pool(name="ct5", bufs=2) as ct5p,
            tc.tile_pool(name="psl0", bufs=2, space="PSUM") as psl0p,
            tc.tile_pool(name="psB", bufs=4, space="PSUM") as psp,
            tc.tile_pool(name="pst", bufs=1, space="PSUM") as pstp,
        ):
            W0_sb = big.tile([P, 26 * 4 * P], bf, name="W0_sb")
            W1_sb = big.tile([P, NCH[1] * SL], bf, name="W1_sb")
            W2_sb = big.tile([P, NCH[2] * SL], bf, name="W2_sb")
            W4_sb = big.tile([P, NCH[4] * SL4], bf, name="W4_sb")
            MSr_sb = big.tile([P, 28], i8, name="MSr_sb")
            MTr_sb = big.tile([P, 28], i8, name="MTr_sb")
            MSc_sb = big.tile([P, 1664], i8, name="MSc_sb")
            MTc_sb = big.tile([P, 1664], i8, name="MTc_sb")
            IDb_sb = big.tile([P, P], bf, name="IDb_sb")
            IDf_sb = big.tile([P, P], f32, name="IDf_sb")
            xt = big.tile([P, 4 * P], bf, name="xt")
            reg0 = big.tile([P, REG], bf, name="reg0")
            reg1 = big.tile([P, REG], bf, name="reg1")
            reg2 = big.tile([P, REG], bf, name="reg2")
            reg3 = big.tile([P, REG], bf, name="reg3")
            reg4 = big.tile([P, L4_N], bf, name="reg4")
            regs = {1: reg1, 2: reg2, 3: reg3, 4: reg4}

            nc.sync.dma_start(W0_sb[:], W0_in[:])
            nc.sync.dma_start(W1_sb[:], W1_in[:])
            nc.sync.dma_start(W2_sb[:], W2_in[:])
            nc.sync.dma_start(W4_sb[:], W4_in[:])
            nc.sync.dma_start(MSr_sb[:], MSr_in[:])
            nc.sync.dma_start(MTr_sb[:], MTr_in[:])
            nc.sync.dma_start(MSc_sb[:], MSc_in[:])
            nc.sync.dma_start(MTc_sb[:], MTc_in[:])
            nc.sync.dma_start(IDb_sb[:], IDb_in[:])
            nc.sync.dma_start(IDf_sb[:], IDf_in[:])
            nc.sync.dma_start(
                xt[:].rearrange("p (c b) -> p c b", b=P),
                xn[:].rearrange("(c p) b -> p c b", p=P))

            def chunk_src(c, r):
                """stationary AP for global source chunk c (0..117)."""
                if c < 4:
                    return xt[:, c * P:(c + 1) * P]
                if c < 108:
                    rg = (reg0, reg1, reg2, reg3)[(c - 4) // 26]
                    cc = (c - 4) % 26
                    return rg[:, cc * P:(cc + 1) * P]
                return reg4[:, (c - 108) * P:(c - 107) * P]

            colls = {}      # (r, l) -> collective instruction
            regnops = {}    # (r, l) -> nop after reg DMAs

            def w3_src(w3t, c):
                if ABLATE_STREAM:
                    return W2_sb[:, :SL]
                return w3t[c // W3_BLK][:, (c % W3_BLK) * SL:(c % W3_BLK + 1) * SL]

            def w5_src(w5t, c):
                if ABLATE_STREAM:
                    return W2_sb[:, :NUM_OUTPUTS]
                return w5t[c // W5_BLK][
                    :, (c % W5_BLK) * NUM_OUTPUTS:(c % W5_BLK + 1) * NUM_OUTPUTS]

            def blayer_mm(r, l, ps_l, c_lo, c_hi, W_src):
                cols = COLS[l]
                for c in range(c_lo, c_hi):
                    nc.tensor.matmul(
                        ps_l[:, :cols],
                        lhsT=chunk_src(c, r),
                        rhs=W_src(c),
                        start=(c == 0), stop=(c == NCH[l] - 1))

            def blayer_finish(r, l, ps_l, f32_out=False):
                """act + col-masks + transposes + bf16 copies (+DMAs)."""
                cols = COLS[l]
                dt = f32 if f32_out else bf
                sfx = "5" if f32_out else ""
                tw = 256 if f32_out else 512
                res = stage.tile([P, tw], dt, name=f"res_{r}_{l}", tag="resB" + sfx)
                sg = stage.tile([P, tw], dt, name=f"sg_{r}_{l}", tag="sgB" + sfx)
                th = stage.tile([P, tw], dt, name=f"th_{r}_{l}", tag="thB" + sfx)
                nc.scalar.activation(res[:, :cols], ps_l[:, :cols], AF.Relu)
                nc.scalar.activation(sg[:, :cols], ps_l[:, :cols], AF.Sigmoid)
                nc.scalar.activation(th[:, :cols], ps_l[:, :cols], AF.Tanh)
                o = COff[l]
                nc.vector.copy_predicated(
                    res[:, :cols], MSc_sb[:, o:o + cols], sg[:, :cols])
                nc.vector.copy_predicated(
                    res[:, :cols], MTc_sb[:, o:o + cols], th[:, :cols])
                dmas = []
                nt = -(-cols // P)
                for t in range(nt):
                    w = min(P, cols - t * P)
                    pt = pstp.tile([P, P], dt, name=f"pt_{r}_{l}_{t}",
                                   tag="pst" + sfx)
                    nc.tensor.transpose(
                        pt[:w, :], res[:, t * P:t * P + w],
                        IDf_sb[:] if f32_out else IDb_sb[:])
                    ct = (ct5p if f32_out else ctp).tile(
                        [P, P], dt, name=f"ct_{r}_{l}_{t}", tag="ct" + sfx)
                    nc.scalar.activation(ct[:w, :], pt[:w, :], AF.Copy)
                    if f32_out:
                        d = nc.sync.dma_start(out[P * t:P * t + w, :], ct[:w, :])
                    else:
                        d = nc.sync.dma_start(
                            contrib[l][P * t:P * t + w, :], ct[:w, :])
                        if (r - 1, l) in colls:
                            add_dep_helper(d.ins, colls[r - 1, l].ins,
                                           reason=f"c{l} WAR")
                    dmas.append(d)
                return dmas

            def do_coll(r, l, dmas):
                if ABLATE_COLL:
                    # timing ablation: replace the AllGather with a local DMA
                    # of the same contrib bytes into g[l] (results are WRONG)
                    n = SL if l < 4 else SL4
                    coll = nc.sync.dma_start(g[l][:n, :], contrib[l][:])
                else:
                    coll = nc.gpsimd.collective_compute(
                        "AllGather", mybir.AluOpType.bypass,
                        replica_groups=[list(range(NCORES))],
                        ins=[contrib[l][:]], outs=[g[l][:]])
                for d in dmas:
                    add_dep_helper(coll.ins, d.ins, reason=f"c{l} RAW")
                if (r - 1, l) in regnops:
                    add_dep_helper(coll.ins, regnops[r - 1, l].ins,
                                   reason=f"g{l} WAR")
                colls[r, l] = coll
                return coll

            def do_reg_dma(r, l, coll):
                n = REG if l < 4 else L4_N
                ds = []
                step = 1024 if l < 4 else L4_N
                for k, lo in enumerate(range(0, n, step)):
                    hi = min(lo + step, n)
                    eng = nc.sync if k % 2 == 0 else nc.scalar
                    d = eng.dma_start(
                        regs[l][:, lo:hi].rearrange("p (c b) -> p c b", b=P),
                        g[l][lo:hi, :].rearrange("(c p) b -> p c b", p=P))
                    add_dep_helper(d.ins, coll.ins, reason=f"g{l} RAW")
                    ds.append(d)
                nop = nc.sync.nop(nofuse=True)
                for d in ds:
                    add_dep_helper(nop.ins, d.ins, reason=f"reg{l} done")
                regnops[r, l] = nop

            def prologue(r):
                """Streams + l0 + l1(+finish+c1) + l2-early for rep r."""
                st = {}
                w3t = []
                w5t = []
                if not ABLATE_STREAM:
                    for b in range(-(-NCH[3] // W3_BLK)):
                        c0, c1_ = b * W3_BLK, min((b + 1) * W3_BLK, NCH[3])
                        wb = w3p.tile([P, W3_BLK * SL], bf, name=f"w3_{r}_{b}", tag="w3")
                        nc.gpsimd.dma_start(wb[:, :(c1_ - c0) * SL],
                                            W3_in[:, c0 * SL:c1_ * SL])
                        w3t.append(wb)
                    for b in range(-(-NCH[5] // W5_BLK)):
                        c0, c1_ = b * W5_BLK, min((b + 1) * W5_BLK, NCH[5])
                        wb = w5p.tile([P, W5_BLK * NUM_OUTPUTS], bf,
                                      name=f"w5_{r}_{b}", tag="w5")
                        nc.gpsimd.dma_start(wb[:, :(c1_ - c0) * NUM_OUTPUTS],
                                            W5_in[:, c0 * NUM_OUTPUTS:c1_ * NUM_OUTPUTS])
                        w5t.append(wb)
                st["w3t"], st["w5t"] = w3t, w5t

                # layer 0: replicated dense, orientation A; dst slots are
                # act-sorted so each chunk needs only range-activations
                B1, B2 = meta["B1"], meta["B2"]
                AFs = [AF.Sigmoid, AF.Tanh, AF.Relu]
                bounds = [0, B1, B2, REG]
                for b in range(7):
                    Wb = BANK_W[b]
                    nsub = Wb // P
                    ps = psl0p.tile([P, 512], f32, name=f"ps0_{r}_{b}", tag="ps0")
                    for s in range(nsub):
                        t = 4 * b + s
                        for c in range(4):
                            nc.tensor.matmul(
                                ps[:, s * P:(s + 1) * P],
                                lhsT=W0_sb[:, (t * 4 + c) * P:(t * 4 + c + 1) * P],
                                rhs=xt[:, c * P:(c + 1) * P],
                                start=(c == 0), stop=(c == 3))
                    for s in range(nsub):
                        t = 4 * b + s
                        lo, hi = t * P, (t + 1) * P
                        cls_lo = 0 if lo < B1 else (1 if lo < B2 else 2)
                        cls_hi = 0 if hi <= B1 else (1 if hi <= B2 else 2)
                        rg = reg0[:, t * P:(t + 1) * P]
                        pss = ps[:, s * P:(s + 1) * P]
                        if cls_lo == cls_hi:
                            nc.scalar.activation(rg, pss, AFs[cls_lo])
                        else:
                            sg = stage.tile([P, P], bf, name=f"sg0_{r}_{b}_{s}",
                                            tag="sg0")
                            th = stage.tile([P, P], bf, name=f"th0_{r}_{b}_{s}",
                                            tag="th0")
                            nc.scalar.activation(rg, pss, AF.Relu)
                            nc.scalar.activation(sg[:], pss, AF.Sigmoid)
                            nc.scalar.activation(th[:], pss, AF.Tanh)
                            nc.vector.copy_predicated(
                                rg, MSr_sb[:, t:t + 1].to_broadcast([P, P]), sg[:])
                            nc.vector.copy_predicated(
                                rg, MTr_sb[:, t:t + 1].to_broadcast([P, P]), th[:])

                # layer 1 (sources x+l0 in SBUF already)
                ps1 = psp.tile([P, 512], f32, name=f"ps1_{r}", tag="ps")
                blayer_mm(r, 1, ps1, 0, 30, lambda c: W1_sb[:, c * SL:(c + 1) * SL])
                st["d1"] = blayer_finish(r, 1, ps1)
                # layer 2 early chunks (x+l0) before COLL1
                ps2 = psp.tile([P, 512], f32, name=f"ps2_{r}", tag="ps")
                blayer_mm(r, 2, ps2, 0, 30, lambda c: W2_sb[:, c * SL:(c + 1) * SL])
                st["ps2"] = ps2
                return st

            st = prologue(0)
            for r in range(reps):
                w3t, w5t = st["w3t"], st["w5t"]
                c1 = do_coll(r, 1, st["d1"])
                do_reg_dma(r, 1, c1)

                # layer 2 late (l1-sourced) + finish
                blayer_mm(r, 2, st["ps2"], 30, 56,
                          lambda c: W2_sb[:, c * SL:(c + 1) * SL])
                d2 = blayer_finish(r, 2, st["ps2"])
                # layer 3 early (x+l0, then l1)
                ps3 = psp.tile([P, 512], f32, name=f"ps3_{r}", tag="ps")
                blayer_mm(r, 3, ps3, 0, 56, lambda c: w3_src(w3t, c))
                c2 = do_coll(r, 2, d2)
                do_reg_dma(r, 2, c2)

                # layer 3 late (l2-sourced) + finish
                blayer_mm(r, 3, ps3, 56, 82, lambda c: w3_src(w3t, c))
                d3 = blayer_finish(r, 3, ps3)
                # layer 4 early (x..l2) ; layer 5 early (x..l2)
                ps4 = psp.tile([P, 512], f32, name=f"ps4_{r}", tag="ps")
                blayer_mm(r, 4, ps4, 0, 82,
                          lambda c: W4_sb[:, c * SL4:(c + 1) * SL4])
                ps5 = psp.tile([P, 512], f32, name=f"ps5_{r}", tag="ps")
                blayer_mm(r, 5, ps5, 0, 82, lambda c: w5_src(w5t, c))
                # ---- software-pipelined prologue of rep r+1: its l0/l1/l2e
                # run on the PE during COLL3/COLL4, so c1(r+1) is ready the
                # moment COLL4(r) completes
                next_st = prologue(r + 1) if r + 1 < reps else None
                c3 = do_coll(r, 3, d3)
                do_reg_dma(r, 3, c3)

                # layer 4 late (l3-sourced) + finish ; layer 5 l3 chunks
                blayer_mm(r, 4, ps4, 82, 108,
                          lambda c: W4_sb[:, c * SL4:(c + 1) * SL4])
                d4 = blayer_finish(r, 4, ps4)
                blayer_mm(r, 5, ps5, 82, 108, lambda c: w5_src(w5t, c))
                c4 = do_coll(r, 4, d4)
                do_reg_dma(r, 4, c4)

                # layer 5 late (l4-sourced) + fp32 finish into out
                blayer_mm(r, 5, ps5, 108, 118, lambda c: w5_src(w5t, c))
                blayer_finish(r, 5, ps5, f32_out=True)
                st = next_st

    split_waits(nc)
    return nc


# ---------------------------------------------------------------- entry point

_CACHE = {}


def _get_compiled(meta, reps=1):
    if reps not in _CACHE:
        _CACHE[reps] = build_nc(meta, reps=reps)
    return _CACHE[reps]


def make_in_maps(x, data):
    xn = np.ascontiguousarray(np.asarray(x, np.float32).T).astype(bf16)
    return [
        {
            "xn": xn,
            "W0": data["W0"], "W1": data["W1"][i], "W2": data["W2"][i],
            "W3": data["W3"][i], "W4": data["W4"][i], "W5": data["W5"],
            "MSr": data["MSr"], "MTr": data["MTr"],
            "MSc": data["MSc"][i], "MTc": data["MTc"][i],
            "IDb": data["IDb"], "IDf": data["IDf"],
        }
        for i in range(NCORES)
    ]


def kernel(x, weights, edge_src, edge_dst, act_id, layer_masks, steps=1, _reps=1):
    meta, data = prep(weights, edge_src, edge_dst, act_id)
    nc = _get_compiled(meta, reps=_reps)
    in_maps = make_in_maps(x, data)
    res = run_bass_kernel_spmd(nc, in_maps, list(range(NCORES)))
    return np.ascontiguousarray(res.results[0]["out"].T.astype(np.float32))



# revision 27
# speedup vs baseline: 3.5813x; 3.5813x over previous
"""Trainium2 Bass kernel v3 for NeatModule forward (gnn_message_passing).

All-dense design, 8 cores SPMD, dst-node sharding with 4 AllGathers/rep:
  - Weights quantized to fp8-e3m4 (x16 scale), ALL SBUF-resident (13.3MB);
    states stay bf16 (mixed-dtype matmuls); activations use scale=1/16.
  - l0 REPLICATED dense, orientation A (dst rows on PSUM partitions).
  - l1..l4 SHARDED dense, orientation B: stationary = source chunk
    [128 src, 128 batch] bf16, moving = W block [128 src, cols] fp8;
    PSUM is [128 batch, cols]. Activations with per-COLUMN masks, then PE
    transposes back to [dst, batch], bf16 copies, contrib DMA, AllGather
    into Shared DRAM, then DMA into the SBUF reg for later stationaries.
  - l5 SHARDED over output columns (32 per core), fused into l4's chunk
    loop (one stationary load per source chunk feeds both ps4 and ps5);
    fp32 finish straight to per-core `out` [32, 128]; host reassembles.
  - contrib/g buffers are double-buffered by rep parity so consecutive
    reps' collectives can be in flight concurrently.
  - Matmuls for chunks whose sources are already available are emitted
    BEFORE the collective they do not depend on, so the PE works through
    each AllGather; reps pipeline through the collective chain.
"""
import numpy as np
import ml_dtypes

import concourse.bass as bass
import concourse.mybir as mybir
import concourse.tile as tile
from concourse.vector_clock import ScopedClock, VectorClock
from concourse.tile_rust import add_dep_helper
from concourse.bass_utils import run_bass_kernel_spmd

NUM_INPUTS = 512
NUM_OUTPUTS = 256
NUM_NODES = 20000
NUM_LAYERS = 6
CHUNK = (NUM_NODES - NUM_INPUTS) // NUM_LAYERS  # 3248
NCORES = 8
P = 128
REG = 3328                   # padded per-layer slot count (l0..l3), 26 chunks
SL = 416                     # per-core dst slots l1..l3
L4_N = 1280                  # l4 compact slots (10 chunks)
SL4 = 160                    # per-core dst slots l4
SL5 = 32                     # per-core output slots l5
L4LO = NUM_INPUTS + 4 * CHUNK    # 13504
L5LO = NUM_INPUTS + 5 * CHUNK    # 16752
OUT0 = NUM_NODES - NUM_OUTPUTS   # 19744
BANK_W = [512] * 6 + [256]       # l0 PSUM bank tiling over 3328 slots
# source chunk counts per dense layer (l1..l5): x,l0 | +l1 | +l2 | +l3
# (l5's l4-sourced part is computed locally per-core and ReduceScattered)
NCH = {1: 30, 2: 56, 3: 82, 4: 108, 5: 108}
COLS = {1: SL, 2: SL, 3: SL, 4: SL4, 5: SL5}
WSCALE = 16.0                # fp8 weight scale; activations divide by it

bf16 = ml_dtypes.bfloat16
f8e3 = ml_dtypes.float8_e3m4
ABLATE_COLL = False

# ---------------------------------------------------------------- tile fixes


def _drain_and_barrier(self, tick_clock, wait_clock):
    gc = tick_clock.global_clock
    n = len(gc)
    for p in range(n):
        t = gc[p]
        if t > 0:
            nop = self.nc.sync.nop(nofuse=True)
            vec = [0] * n
            vec[p] = t
            wait_clock.add_sem_waits(nop.ins, ScopedClock({None: VectorClock(vec)}))
    self.nc.sync.drain()
    self.nc.all_engine_barrier()
    assert self.sems is not None
    popped = self.nc._tile_sem_poison_stack.pop()
    assert popped is self._sem_poison
    self.nc.clear_and_free_semaphores(list(self.sems.allocated().values()))
    self.nc.all_engine_barrier()


tile.TileContext._drain_and_barrier = _drain_and_barrier


def split_waits(nc, K=1):
    n_split = 0
    for f in nc.m.functions:
        for bb in f.blocks:
            insts = list(bb.instructions)
            out = []
            changed = False
            for inst in insts:
                si = inst.sync_info
                if si is not None and si.on_wait is not None and len(si.on_wait) > K:
                    waits = list(si.on_wait)
                    over, keep = waits[:-K], waits[-K:]
                    for j in range(0, len(over), K):
                        out.append(mybir.InstNoOp(
                            name=f"I-waitsplit-{nc.next_id()}",
                            engine=inst.engine,
                            sync_info=mybir.SyncInfo(on_wait=over[j:j + K], on_update=[]),
                        ))
                    si.on_wait = keep
                    inst.sync_info = si
                    changed = True
                    n_split += 1
                out.append(inst)
            if changed:
                bb.instructions = out
    return n_split


# ------------------------------------------------------------------ host prep


def _prune_edges(edge_src, edge_dst):
    needed = np.zeros(NUM_NODES, dtype=bool)
    needed[NUM_NODES - NUM_OUTPUTS:] = True
    layer_of = np.full(NUM_NODES, -1)
    for l in range(NUM_LAYERS):
        layer_of[NUM_INPUTS + l * CHUNK: NUM_INPUTS + (l + 1) * CHUNK] = l
    dst_l = layer_of[edge_dst]
    keep = np.zeros(len(edge_src), dtype=bool)
    for l in range(NUM_LAYERS - 1, -1, -1):
        m = (dst_l == l) & needed[edge_dst]
        keep |= m
        needed[edge_src[m]] = True
    return keep, needed


def _q8(a):
    """fp32 -> fp8 e3m4 with x16 scale."""
    return (np.asarray(a, np.float32) * WSCALE).astype(f8e3)


def prep(weights, edge_src, edge_dst, act_id):
    es = np.asarray(edge_src).astype(np.int64)
    ed = np.asarray(edge_dst).astype(np.int64)
    ew = np.asarray(weights).astype(np.float32)
    act_id = np.asarray(act_id).astype(np.int64)

    keep, needed = _prune_edges(es, ed)
    es, ed, ew = es[keep], ed[keep], ew[keep]
    e_l = (ed - NUM_INPUTS) // CHUNK

    needed4 = np.where(needed[L4LO:L5LO])[0]      # rel ids of kept l4 nodes
    n4 = len(needed4)
    assert n4 <= L4_N
    c4map = np.full(CHUNK, -1, dtype=np.int64)
    c4map[needed4] = np.arange(n4)

    # l0 slot order sorted by activation class (sig | tanh | relu+pads) so the
    # replicated l0 needs only range-activations, no predicated copies.
    act0 = act_id[NUM_INPUTS:NUM_INPUTS + CHUNK]
    cls0 = np.full(REG, 2, dtype=np.int64)
    cls0[:CHUNK] = np.where(act0 == 0, 0, np.where(act0 == 1, 1, 2))
    perm0 = np.lexsort((np.arange(REG), cls0))    # slot s holds rel perm0[s]
    inv0 = np.empty(REG, dtype=np.int64)
    inv0[perm0] = np.arange(REG)
    B1 = int((cls0 == 0).sum())
    B2 = B1 + int((cls0 == 1).sum())

    # global source-chunk index of a source node (chunk-structured regions)
    def src_chunk_pos(s):
        """-> (chunk index 0..117, row within chunk)"""
        gc = np.where(s < NUM_INPUTS, s // P, 0)
        pp = np.where(s < NUM_INPUTS, s % P, 0)
        for l in range(4):
            lo = NUM_INPUTS + l * CHUNK
            m = (s >= lo) & (s < lo + CHUNK)
            rel = np.clip(s - lo, 0, CHUNK - 1)
            pos = inv0[rel] if l == 0 else rel
            gc = np.where(m, 4 + 26 * l + pos // P, gc)
            pp = np.where(m, pos % P, pp)
        m4 = (s >= L4LO) & (s < L5LO)
        if m4.any():
            cc = c4map[np.clip(s - L4LO, 0, CHUNK - 1)]
            assert (cc[m4] >= 0).all()
            gc = np.where(m4, 108 + cc // P, gc)
            pp = np.where(m4, cc % P, pp)
        return gc.astype(np.int64), pp.astype(np.int64)

    # ---------------- W0 (replicated, orientation A, act-sorted dst slots)
    W0 = np.zeros((P, 26 * 4 * P), dtype=np.float32)
    m = e_l == 0
    s, w = es[m], ew[m]
    d = inv0[ed[m] - NUM_INPUTS]
    c, sp = s // P, s % P
    t, q = d // P, d % P
    np.add.at(W0, (sp, (t * 4 + c) * P + q), w)

    # ---------------- W1..W4 (sharded, orientation B: [src_p, chunk*cols + dstcol])
    Wd = {}
    for l in (1, 2, 3, 4):
        cols = COLS[l]
        Wl = np.zeros((NCORES, P, NCH[l] * cols), dtype=np.float32)
        m = e_l == l
        s, w = es[m], ew[m]
        if l < 4:
            dd = ed[m] - (NUM_INPUTS + l * CHUNK)
        else:
            dd = c4map[ed[m] - L4LO]
            assert (dd >= 0).all()
        cor = dd // cols
        j = dd % cols
        gc, sp = src_chunk_pos(s)
        assert gc.max() < NCH[l]
        np.add.at(Wl, (cor, sp, gc * cols + j), w)
        Wd[l] = Wl

    # ---------------- W5 (sharded over output columns, orientation B) for
    # x..l3 sources; l4-sourced edges go to W5L4 (owner-core local partial)
    W5 = np.zeros((NCORES, P, NCH[5] * SL5), dtype=np.float32)
    W5L4 = np.zeros((NCORES, P, 2 * NUM_OUTPUTS), dtype=np.float32)
    m = e_l == 5
    s, w = es[m], ew[m]
    oc = ed[m] - OUT0
    assert (oc >= 0).all()
    gc, sp = src_chunk_pos(s)
    early = gc < NCH[5]
    np.add.at(W5, ((oc // SL5)[early], sp[early],
                   gc[early] * SL5 + (oc % SL5)[early]), w[early])
    late = ~early
    cc = c4map[s[late] - L4LO]
    assert (cc >= 0).all()
    own = cc // SL4
    rel = cc - own * SL4
    np.add.at(W5L4, (own, rel % P, (rel // P) * NUM_OUTPUTS + oc[late]), w[late])

    # ---------------- masks
    rows = np.arange(P)
    # row masks (l0: 26 cols), replicated
    MSr = np.zeros((P, 26), dtype=np.int8)
    MTr = np.zeros((P, 26), dtype=np.int8)
    for t in range(26):
        rel_s = perm0[t * P + rows]          # act-sorted slot order
        v = rel_s < CHUNK
        nd = NUM_INPUTS + rel_s[v]
        MSr[rows[v], t] = act_id[nd] == 0
        MTr[rows[v], t] = act_id[nd] == 1
    # column masks per core (l1..l5): value per dst col, broadcast down rows;
    # layout [416 | 416 | 416 | 160 | 32] = 1472 cols
    MSc = np.zeros((NCORES, P, 1472), dtype=np.int8)
    MTc = np.zeros((NCORES, P, 1472), dtype=np.int8)
    off = {1: 0, 2: 416, 3: 832, 4: 1248, 5: 1408}
    for l in (1, 2, 3):
        base = NUM_INPUTS + l * CHUNK
        for i in range(NCORES):
            jj = np.arange(SL)
            rel = SL * i + jj
            v = rel < CHUNK
            nd = base + rel[v]
            MSc[i, :, off[l] + jj[v]] = (act_id[nd] == 0).astype(np.int8)[:, None]
            MTc[i, :, off[l] + jj[v]] = (act_id[nd] == 1).astype(np.int8)[:, None]
    for i in range(NCORES):
        jj = np.arange(SL4)
        cc = SL4 * i + jj
        v = cc < n4
        nd = L4LO + needed4[cc[v]]
        MSc[i, :, off[4] + jj[v]] = (act_id[nd] == 0).astype(np.int8)[:, None]
        MTc[i, :, off[4] + jj[v]] = (act_id[nd] == 1).astype(np.int8)[:, None]
    jj = np.arange(SL5)
    for i in range(NCORES):
        nd = OUT0 + SL5 * i + jj
        MSc[i, :, off[5] + jj] = (act_id[nd] == 0).astype(np.int8)[:, None]
        MTc[i, :, off[5] + jj] = (act_id[nd] == 1).astype(np.int8)[:, None]

    ident = np.eye(P, dtype=np.float32)

    data = dict(
        W0=_q8(W0),
        W1=_q8(Wd[1]), W2=_q8(Wd[2]),
        W3=_q8(Wd[3]), W4=_q8(Wd[4]),
        W5=_q8(W5), W5L4=_q8(W5L4),
        MSr=MSr, MTr=MTr, MSc=MSc, MTc=MTc,
        IDb=ident.astype(bf16), IDf=ident,
    )
    return dict(B1=B1, B2=B2), data


# -------------------------------------------------------------- kernel build


def build_nc(meta, reps=1):
    f32 = mybir.dt.float32
    bf = mybir.dt.bfloat16
    f8 = mybir.dt.float8e3
    i8 = mybir.dt.int8
    AF = mybir.ActivationFunctionType
    ISCALE = 1.0 / WSCALE

    nc = bass.Bass()
    xn = nc.declare_dram_parameter("xn", [NUM_INPUTS, P], bf, isOutput=False)
    W0_in = nc.declare_dram_parameter("W0", [P, 26 * 4 * P], f8, isOutput=False)
    W1_in = nc.declare_dram_parameter("W1", [P, NCH[1] * SL], f8, isOutput=False)
    W2_in = nc.declare_dram_parameter("W2", [P, NCH[2] * SL], f8, isOutput=False)
    W3_in = nc.declare_dram_parameter("W3", [P, NCH[3] * SL], f8, isOutput=False)
    W4_in = nc.declare_dram_parameter("W4", [P, NCH[4] * SL4], f8, isOutput=False)
    W5_in = nc.declare_dram_parameter("W5", [P, NCH[5] * SL5], f8, isOutput=False)
    W5L4_in = nc.declare_dram_parameter("W5L4", [P, 2 * NUM_OUTPUTS], f8,
                                        isOutput=False)
    MSr_in = nc.declare_dram_parameter("MSr", [P, 26], i8, isOutput=False)
    MTr_in = nc.declare_dram_parameter("MTr", [P, 26], i8, isOutput=False)
    MSc_in = nc.declare_dram_parameter("MSc", [P, 1472], i8, isOutput=False)
    MTc_in = nc.declare_dram_parameter("MTc", [P, 1472], i8, isOutput=False)
    IDb_in = nc.declare_dram_parameter("IDb", [P, P], bf, isOutput=False)
    IDf_in = nc.declare_dram_parameter("IDf", [P, P], f32, isOutput=False)
    out = nc.declare_dram_parameter("out", [SL5, P], f32, isOutput=True)

    # double-buffered by rep parity so consecutive reps' collectives overlap
    g = {(l, pr): nc.dram_tensor(f"g{l}_{pr}", [REG, P], bf, addr_space="Shared")
         for l in (1, 2, 3) for pr in (0, 1)}
    contrib = {(l, pr): nc.dram_tensor(f"c{l}_{pr}", [SL, P], bf)
               for l in (1, 2, 3) for pr in (0, 1)}
    c5p = {pr: nc.dram_tensor(f"c5p_{pr}", [NUM_OUTPUTS, P], bf)
           for pr in (0, 1)}
    rs5 = {pr: nc.dram_tensor(f"rs5_{pr}", [SL5, P], bf) for pr in (0, 1)}

    COff = {1: 0, 2: 416, 3: 832, 4: 1248, 5: 1408}

    with tile.TileContext(nc) as tc:
        with (
            tc.tile_pool(name="big", bufs=1) as big,
            tc.tile_pool(name="stage", bufs=2) as stage,
            tc.tile_pool(name="ct", bufs=6) as ctp,
            tc.tile_pool(name="ct5", bufs=2) as ct5p,
            tc.tile_pool(name="l4loc", bufs=2) as l4locp,
            tc.tile_pool(name="psl0", bufs=2, space="PSUM") as psl0p,
            tc.tile_pool(name="psB", bufs=4, space="PSUM") as psp,
            tc.tile_pool(name="pst", bufs=1, space="PSUM") as pstp,
        ):
            W_sb = {}
            W_sb[0] = big.tile([P, 26 * 4 * P], f8, name="W0_sb")
            W_sb[1] = big.tile([P, NCH[1] * SL], f8, name="W1_sb")
            W_sb[2] = big.tile([P, NCH[2] * SL], f8, name="W2_sb")
            W_sb[3] = big.tile([P, NCH[3] * SL], f8, name="W3_sb")
            W_sb[4] = big.tile([P, NCH[4] * SL4], f8, name="W4_sb")
            W_sb[5] = big.tile([P, NCH[5] * SL5], f8, name="W5_sb")
            W5L4_sb = big.tile([P, 2 * NUM_OUTPUTS], f8, name="W5L4_sb")
            MSr_sb = big.tile([P, 26], i8, name="MSr_sb")
            MTr_sb = big.tile([P, 26], i8, name="MTr_sb")
            MSc_sb = big.tile([P, 1472], i8, name="MSc_sb")
            MTc_sb = big.tile([P, 1472], i8, name="MTc_sb")
            IDb_sb = big.tile([P, P], bf, name="IDb_sb")
            IDf_sb = big.tile([P, P], f32, name="IDf_sb")
            xt = big.tile([P, 4 * P], bf, name="xt")
            reg0 = big.tile([P, REG], bf, name="reg0")
            reg1 = big.tile([P, REG], bf, name="reg1")
            reg2 = big.tile([P, REG], bf, name="reg2")
            reg3 = big.tile([P, REG], bf, name="reg3")
            reg4 = big.tile([P, L4_N], bf, name="reg4")
            regs = {1: reg1, 2: reg2, 3: reg3, 4: reg4}

            for l, win in ((0, W0_in), (1, W1_in), (2, W2_in),
                           (3, W3_in), (4, W4_in), (5, W5_in)):
                nc.sync.dma_start(W_sb[l][:], win[:])
            nc.sync.dma_start(W5L4_sb[:], W5L4_in[:])
            nc.sync.dma_start(MSr_sb[:], MSr_in[:])
            nc.sync.dma_start(MTr_sb[:], MTr_in[:])
            nc.sync.dma_start(MSc_sb[:], MSc_in[:])
            nc.sync.dma_start(MTc_sb[:], MTc_in[:])
            nc.sync.dma_start(IDb_sb[:], IDb_in[:])
            nc.sync.dma_start(IDf_sb[:], IDf_in[:])
            nc.sync.dma_start(
                xt[:].rearrange("p (c b) -> p c b", b=P),
                xn[:].rearrange("(c p) b -> p c b", p=P))

            def chunk_src(c):
                """stationary AP for global source chunk c (0..117)."""
                if c < 4:
                    return xt[:, c * P:(c + 1) * P]
                if c < 108:
                    rg = (reg0, reg1, reg2, reg3)[(c - 4) // 26]
                    cc = (c - 4) % 26
                    return rg[:, cc * P:(cc + 1) * P]
                return reg4[:, (c - 108) * P:(c - 107) * P]

            colls = {}      # (r, l) -> collective instruction
            regnops = {}    # (r, l) -> nop after reg DMAs
            rs5colls = {}   # r -> ReduceScatter instruction
            rs5nops = {}    # r -> nop after rs5 readback

            def blayer_mm(l, ps_l, c_lo, c_hi):
                cols = COLS[l]
                for c in range(c_lo, c_hi):
                    nc.tensor.matmul(
                        ps_l[:, :cols],
                        lhsT=chunk_src(c),
                        rhs=W_sb[l][:, c * cols:(c + 1) * cols],
                        start=(c == 0), stop=(c == NCH[l] - 1))

            def blayer_finish(r, l, ps_l, f32_out=False, local_tile=None):
                """act(scale) + col-masks + transposes + bf16 copies (+DMAs)."""
                cols = COLS[l]
                dt = f32 if f32_out else bf
                sfx = "5" if f32_out else ""
                tw = 32 if f32_out else 512
                res = stage.tile([P, tw], dt, name=f"res_{r}_{l}", tag="resB" + sfx)
                sg = stage.tile([P, tw], dt, name=f"sg_{r}_{l}", tag="sgB" + sfx)
                th = stage.tile([P, tw], dt, name=f"th_{r}_{l}", tag="thB" + sfx)
                nc.scalar.activation(res[:, :cols], ps_l[:, :cols], AF.Relu,
                                     scale=ISCALE)
                nc.scalar.activation(sg[:, :cols], ps_l[:, :cols], AF.Sigmoid,
                                     scale=ISCALE)
                nc.scalar.activation(th[:, :cols], ps_l[:, :cols], AF.Tanh,
                                     scale=ISCALE)
                o = COff[l]
                nc.vector.copy_predicated(
                    res[:, :cols], MSc_sb[:, o:o + cols], sg[:, :cols])
                nc.vector.copy_predicated(
                    res[:, :cols], MTc_sb[:, o:o + cols], th[:, :cols])
                dmas = []
                nt = -(-cols // P)
                for t in range(nt):
                    w = min(P, cols - t * P)
                    pt = pstp.tile([P, P], dt, name=f"pt_{r}_{l}_{t}",
                                   tag="pst" + sfx)
                    nc.tensor.transpose(
                        pt[:w, :], res[:, t * P:t * P + w],
                        IDf_sb[:] if f32_out else IDb_sb[:])
                    if local_tile is not None:
                        nc.scalar.activation(local_tile[:w, t * P:(t + 1) * P],
                                             pt[:w, :], AF.Copy)
                        continue
                    ct = (ct5p if f32_out else ctp).tile(
                        [P, P], dt, name=f"ct_{r}_{l}_{t}", tag="ct" + sfx)
                    nc.scalar.activation(ct[:w, :], pt[:w, :], AF.Copy)
                    if f32_out:
                        d = nc.sync.dma_start(out[P * t:P * t + w, :], ct[:w, :])
                    else:
                        d = nc.sync.dma_start(
                            contrib[l, r % 2][P * t:P * t + w, :], ct[:w, :])
                        if (r - 2, l) in colls:
                            add_dep_helper(d.ins, colls[r - 2, l].ins,
                                           reason=f"c{l} WAR")
                    dmas.append(d)
                return dmas

            def do_coll(r, l, dmas):
                if ABLATE_COLL:
                    # timing ablation: replace the AllGather with a local DMA
                    # of the same contrib bytes into g[l] (results are WRONG)
                    n = SL if l < 4 else SL4
                    coll = nc.sync.dma_start(g[l, r % 2][:n, :], contrib[l, r % 2][:])
                else:
                    coll = nc.gpsimd.collective_compute(
                        "AllGather", mybir.AluOpType.bypass,
                        replica_groups=[list(range(NCORES))],
                        ins=[contrib[l, r % 2][:]], outs=[g[l, r % 2][:]])
                for d in dmas:
                    add_dep_helper(coll.ins, d.ins, reason=f"c{l} RAW")
                if (r - 2, l) in regnops:
                    add_dep_helper(coll.ins, regnops[r - 2, l].ins,
                                   reason=f"g{l} WAR")
                colls[r, l] = coll
                return coll

            def do_reg_dma(r, l, coll):
                n = REG if l < 4 else L4_N
                ds = []
                step = 1024 if l < 4 else L4_N
                for k, lo in enumerate(range(0, n, step)):
                    hi = min(lo + step, n)
                    eng = nc.sync if k % 2 == 0 else nc.scalar
                    d = eng.dma_start(
                        regs[l][:, lo:hi].rearrange("p (c b) -> p c b", b=P),
                        g[l, r % 2][lo:hi, :].rearrange("(c p) b -> p c b", p=P))
                    add_dep_helper(d.ins, coll.ins, reason=f"g{l} RAW")
                    ds.append(d)
                nop = nc.sync.nop(nofuse=True)
                for d in ds:
                    add_dep_helper(nop.ins, d.ins, reason=f"reg{l} done")
                regnops[r, l] = nop

            def l0_banks(r, b_lo, b_hi):
                """layer 0 (replicated, orientation A) for bank groups
                [b_lo, b_hi); dst slots act-sorted -> range activations."""
                B1, B2 = meta["B1"], meta["B2"]
                AFs = [AF.Sigmoid, AF.Tanh, AF.Relu]
                for b in range(b_lo, b_hi):
                    Wb = BANK_W[b]
                    nsub = Wb // P
                    ps = psl0p.tile([P, 512], f32, name=f"ps0_{r}_{b}", tag="ps0")
                    for s in range(nsub):
                        t = 4 * b + s
                        for c in range(4):
                            nc.tensor.matmul(
                                ps[:, s * P:(s + 1) * P],
                                lhsT=W_sb[0][:, (t * 4 + c) * P:(t * 4 + c + 1) * P],
                                rhs=xt[:, c * P:(c + 1) * P],
                                start=(c == 0), stop=(c == 3))
                    for s in range(nsub):
                        t = 4 * b + s
                        lo, hi = t * P, (t + 1) * P
                        cls_lo = 0 if lo < B1 else (1 if lo < B2 else 2)
                        cls_hi = 0 if hi <= B1 else (1 if hi <= B2 else 2)
                        rg = reg0[:, t * P:(t + 1) * P]
                        pss = ps[:, s * P:(s + 1) * P]
                        if cls_lo == cls_hi:
                            nc.scalar.activation(rg, pss, AFs[cls_lo], scale=ISCALE)
                        else:
                            sg = stage.tile([P, P], bf, name=f"sg0_{r}_{b}_{s}",
                                            tag="sg0")
                            th = stage.tile([P, P], bf, name=f"th0_{r}_{b}_{s}",
                                            tag="th0")
                            nc.scalar.activation(rg, pss, AF.Relu, scale=ISCALE)
                            nc.scalar.activation(sg[:], pss, AF.Sigmoid, scale=ISCALE)
                            nc.scalar.activation(th[:], pss, AF.Tanh, scale=ISCALE)
                            nc.vector.copy_predicated(
                                rg, MSr_sb[:, t:t + 1].to_broadcast([P, P]), sg[:])
                            nc.vector.copy_predicated(
                                rg, MTr_sb[:, t:t + 1].to_broadcast([P, P]), th[:])

            def prologue_l1(r):
                """l1 (+finish -> d1); runs after l0."""
                st = {}
                ps1 = psp.tile([P, 512], f32, name=f"ps1_{r}", tag="ps")
                blayer_mm(1, ps1, 0, 30)
                st["d1"] = blayer_finish(r, 1, ps1)
                return st

            def prologue_l2e(r):
                ps2 = psp.tile([P, 512], f32, name=f"ps2_{r}", tag="ps")
                blayer_mm(2, ps2, 0, 30)
                return ps2

            def fused345(ps3, ps4, ps5, c_lo, c_hi):
                """one stationary load per source chunk feeds l3, l4 and l5."""
                for c in range(c_lo, c_hi):
                    src = chunk_src(c)
                    nc.tensor.matmul(
                        ps3[:, :SL], lhsT=src,
                        rhs=W_sb[3][:, c * SL:(c + 1) * SL],
                        start=(c == 0), stop=(c == NCH[3] - 1))
                    nc.tensor.matmul(
                        ps4[:, :SL4], lhsT=src,
                        rhs=W_sb[4][:, c * SL4:(c + 1) * SL4],
                        start=(c == 0), stop=False)
                    nc.tensor.matmul(
                        ps5[:, :SL5], lhsT=src,
                        rhs=W_sb[5][:, c * SL5:(c + 1) * SL5],
                        start=(c == 0), stop=False)

            def fused45(ps4, ps5, c_lo, c_hi):
                for c in range(c_lo, c_hi):
                    src = chunk_src(c)
                    nc.tensor.matmul(
                        ps4[:, :SL4], lhsT=src,
                        rhs=W_sb[4][:, c * SL4:(c + 1) * SL4],
                        start=False, stop=(c == 107))
                    nc.tensor.matmul(
                        ps5[:, :SL5], lhsT=src,
                        rhs=W_sb[5][:, c * SL5:(c + 1) * SL5],
                        start=False, stop=(c == NCH[5] - 1))

            # ---- software-pipelined steady-state loop: every collective is
            # followed (in PE program order) by matmul work that does not
            # depend on it, drawn from this rep's later layers or the next
            # rep's l0/l1/l2e; c1 of rep r+1 is issued one body early so it
            # is fully hidden behind rep r's tail.
            l0_banks(0, 0, 7)
            st = prologue_l1(0)
            st["c1"] = do_coll(0, 1, st["d1"])
            do_reg_dma(0, 1, st["c1"])
            st["ps2"] = prologue_l2e(0)
            for r in range(reps):
                # fused l3+l4+l5 chunks 0..30 (x+l0 sources): tail cover of c1
                ps3 = psp.tile([P, 512], f32, name=f"ps3_{r}", tag="ps")
                ps4 = psp.tile([P, 512], f32, name=f"ps4_{r}", tag="ps")
                ps5 = psp.tile([P, 512], f32, name=f"ps5_{r}", tag="ps")
                fused345(ps3, ps4, ps5, 0, 30)
                # dependent on reg1: layer 2 late + finish
                blayer_mm(2, st["ps2"], 30, 56)
                d2 = blayer_finish(r, 2, st["ps2"])
                c2 = do_coll(r, 2, d2)
                do_reg_dma(r, 2, c2)
                # cover c2: fused chunks 30..56 (l1) + next rep's l0 banks 0..3
                fused345(ps3, ps4, ps5, 30, 56)
                if r + 1 < reps:
                    l0_banks(r + 1, 0, 3)
                # dependent on reg2: fused chunks 56..82 + l3 finish
                fused345(ps3, ps4, ps5, 56, 82)
                d3 = blayer_finish(r, 3, ps3)
                c3 = do_coll(r, 3, d3)
                do_reg_dma(r, 3, c3)
                # cover c3: next rep's l0 banks 3..7, l1 (+d1) and its c1
                next_st = None
                if r + 1 < reps:
                    l0_banks(r + 1, 3, 7)
                    next_st = prologue_l1(r + 1)
                # dependent on reg3: fused l4+l5 late; l4 finishes into a
                # LOCAL stationary (no gather) feeding this core's partial
                # contribution to all 256 outputs, which is ReduceScattered.
                fused45(ps4, ps5, 82, 108)
                # free ps5's bank early: its x..l3 accumulation is complete
                s5e = stage.tile([P, SL5], f32, name=f"s5e_{r}", tag="s5e")
                nc.scalar.activation(s5e[:], ps5[:, :SL5], AF.Copy)
                l4t = l4locp.tile([P, 2 * P], bf, name=f"l4t_{r}", tag="l4t")
                blayer_finish(r, 4, ps4, local_tile=l4t)
                ps5p = psl0p.tile([P, 512], f32, name=f"ps5p_{r}", tag="ps0")
                nc.tensor.matmul(
                    ps5p[:, :NUM_OUTPUTS], lhsT=l4t[:, 0:P],
                    rhs=W5L4_sb[:, 0:NUM_OUTPUTS], start=True, stop=False)
                nc.tensor.matmul(
                    ps5p[:, :NUM_OUTPUTS], lhsT=l4t[0:SL4 - P, P:2 * P],
                    rhs=W5L4_sb[0:SL4 - P, NUM_OUTPUTS:2 * NUM_OUTPUTS],
                    start=False, stop=True)
                s5p = stage.tile([P, NUM_OUTPUTS], bf, name=f"s5p_{r}", tag="s5p")
                nc.scalar.activation(s5p[:], ps5p[:, :NUM_OUTPUTS], AF.Copy)
                dmas5 = []
                for t in range(2):
                    pt = pstp.tile([P, P], bf, name=f"pt5p_{r}_{t}", tag="pst")
                    nc.tensor.transpose(pt[:], s5p[:, t * P:(t + 1) * P],
                                        IDb_sb[:])
                    ct = ctp.tile([P, P], bf, name=f"ct5p_{r}_{t}", tag="ct")
                    nc.scalar.activation(ct[:], pt[:], AF.Copy)
                    d = nc.sync.dma_start(c5p[r % 2][P * t:P * (t + 1), :], ct[:])
                    if (r - 2) in rs5colls:
                        add_dep_helper(d.ins, rs5colls[r - 2].ins,
                                       reason="c5p WAR")
                    dmas5.append(d)
                if ABLATE_COLL:
                    rs = nc.sync.dma_start(rs5[r % 2][:], c5p[r % 2][:SL5, :])
                else:
                    rs = nc.gpsimd.collective_compute(
                        "ReduceScatter", mybir.AluOpType.add,
                        replica_groups=[list(range(NCORES))],
                        ins=[c5p[r % 2][:]], outs=[rs5[r % 2][:]])
                for d in dmas5:
                    add_dep_helper(rs.ins, d.ins, reason="c5p RAW")
                if (r - 2) in rs5nops:
                    add_dep_helper(rs.ins, rs5nops[r - 2].ins, reason="rs5 WAR")
                rs5colls[r] = rs
                # next rep's c1 queues right behind the (small) rs5
                if r + 1 < reps:
                    next_st["c1"] = do_coll(r + 1, 1, next_st["d1"])
                    do_reg_dma(r + 1, 1, next_st["c1"])
                # cover rs5: next rep's l2-early
                if r + 1 < reps:
                    next_st["ps2"] = prologue_l2e(r + 1)
                # dependent on rs5: read back, transpose, add to ps5, finish
                rsb = stage.tile([SL5, P], bf, name=f"rsb_{r}", tag="rsb")
                drs = nc.scalar.dma_start(rsb[:], rs5[r % 2][:])
                add_dep_helper(drs.ins, rs.ins, reason="rs5 RAW")
                rnop = nc.sync.nop(nofuse=True)
                add_dep_helper(rnop.ins, drs.ins, reason="rsb done")
                rs5nops[r] = rnop
                ptr = pstp.tile([P, P], bf, name=f"ptr_{r}", tag="pst5")
                nc.tensor.transpose(ptr[:, :SL5], rsb[:], IDb_sb[:SL5, :SL5])
                s5sum = stage.tile([P, SL5], f32, name=f"s5sum_{r}", tag="s5sum")
                nc.vector.tensor_add(s5sum[:], s5e[:], ptr[:, :SL5])
                blayer_finish(r, 5, s5sum, f32_out=True)
                st = next_st

    split_waits(nc)
    return nc


# ---------------------------------------------------------------- entry point

_CACHE = {}


def _get_compiled(meta, reps=1):
    if reps not in _CACHE:
        _CACHE[reps] = build_nc(meta, reps=reps)
    return _CACHE[reps]


def make_in_maps(x, data):
    xn = np.ascontiguousarray(np.asarray(x, np.float32).T).astype(bf16)
    return [
        {
            "xn": xn,
            "W0": data["W0"], "W1": data["W1"][i], "W2": data["W2"][i],
            "W3": data["W3"][i], "W4": data["W4"][i], "W5": data["W5"][i],
            "W5L4": data["W5L4"][i],
            "MSr": data["MSr"], "MTr": data["MTr"],
            "MSc": data["MSc"][i], "MTc": data["MTc"][i],
            "IDb": data["IDb"], "IDf": data["IDf"],
        }
        for i in range(NCORES)
    ]


def assemble(results):
    """[{'out': [32,128]} per core] -> full [128, 256] output."""
    full = np.empty((P, NUM_OUTPUTS), dtype=np.float32)
    for i in range(NCORES):
        full[:, SL5 * i:SL5 * (i + 1)] = results[i]["out"].T.astype(np.float32)
    return np.ascontiguousarray(full)


def kernel(x, weights, edge_src, edge_dst, act_id, layer_masks, steps=1, _reps=1):
    meta, data = prep(weights, edge_src, edge_dst, act_id)
    nc = _get_compiled(meta, reps=_reps)
    in_maps = make_in_maps(x, data)
    res = run_bass_kernel_spmd(nc, in_maps, list(range(NCORES)))
    return assemble(res.results)
